# revision 3
# baseline (speedup 1.0000x reference)
"""Trainium2 Bass kernel for DecomposedQValueNN (gnn_message_passing).

Per batch row b of x[65536, 128]:
  xa = x.reshape(B, 32, 4); other_a = MLP_o(xa[:,a]) (3 relu layers, 4-32-32-16)
  sum_other = sum_{a != sel} other_a;  sel_out = MLP_s(xa[:,sel])
  h = relu([sel_out; sum_other] @ gW1 + gb1); q = h @ gW2 + gb2
  out[b] = q[b, clip(int(xa[b,sel,3]),0,1)]

Device mapping (8 cores, pure batch data-parallel, 8192 rows/core):
  activations transposed [feat, batch]; PE 32x32 array tiling runs 16
  per-agent matmuls concurrently; bias+relu fused into PSUM->SBUF
  evacuation alternating between ScalarE and VectorE; masked agent sum via
  ones-matmuls; sel-MLP + global head col-tiled 4 chunks at a time.
  Final 2-way q gather on host.
"""

import numpy as np

B_FULL = 65536
N_CORES = 8
B_C = B_FULL // N_CORES       # 8192
A, D = 32, 4
NCH = 512                     # batch cols per chunk (1 PSUM bank fp32)
CHUNKS = B_C // NCH           # 16
SP = 4                        # chunks per superpass

# wpack column offsets
OW1BD, OW2, OW3, OSW1, OSW2, OSW3 = 0, 256, 288, 320, 352, 384
OONF, OONE, OGSEL, OGSUM, OGW2, OIDN = 416, 448, 480, 512, 544, 576
OB1, OB2, OB3, OSB1, OSB2, OSB3, OGB1, OGB2 = 704, 705, 706, 707, 708, 709, 710, 711
WCOLS = 712

_COMPILED = {}

# test-harness knobs (harness leaves these at defaults)
TRACE = False
TRACE_KW = {}
LAST_RESULT = None


def _f32(a):
    return np.ascontiguousarray(a, dtype=np.float32)


def _build_wpack(sel, oW1, ob1, oW2, ob2, oW3, ob3,
                 sW1, sb1, sW2, sb2, sW3, sb3, gW1, gb1, gW2, gb2):
    P = 128

    def rep4(w):              # [32, m] -> [128, m] per row-group
        return np.tile(_f32(w), (4, 1))

    def repD(w):              # [4, m] -> [128, m] every 4 rows
        return np.tile(_f32(w), (32, 1))

    def padc(w, m):
        w = _f32(w)
        return np.concatenate([w, np.zeros((w.shape[0], m - w.shape[1]), np.float32)], 1)

    # L1 weights as zero-padded K=32 blocks: block u has oW1 at rows 4u..4u+3
    w1bd = np.zeros((32, 8 * 32), np.float32)
    for u in range(8):
        w1bd[4 * u:4 * u + 4, 32 * u:32 * u + 32] = _f32(oW1)
    w1 = np.tile(w1bd, (4, 1))
    w2 = rep4(oW2)
    w3 = rep4(padc(oW3, 32))
    u_sel = sel % 8
    sw1p = np.zeros((32, 32), np.float32)
    sw1p[4 * u_sel:4 * u_sel + 4, :] = _f32(sW1)
    sw1 = np.tile(sw1p, (4, 1))
    sw2 = rep4(sW2)
    sw3 = rep4(padc(sW3, 32))

    # ones matrices for agent sum: out[m] = sum_p ones[p,m] * z3s[p]
    # z3s tile (p,i): agent a = 8i+4p+j at rows 32j..32j+15
    def ones_mat(excl_j):
        o = np.zeros((P, 32), np.float32)
        for j in range(4):
            if j == excl_j:
                continue
            for m in range(16):
                o[32 * j + m, m] = 1.0
        return o

    sel_i, sel_p, sel_j = sel // 8, (sel % 8) // 4, sel % 4
    ones_full = ones_mat(-1)
    ones_excl = ones_mat(sel_j)

    gsel = np.zeros((P, 32), np.float32)
    gsum = np.zeros((P, 32), np.float32)
    for c in range(4):
        gsel[32 * c:32 * c + 16, :] = _f32(gW1)[0:16, :]
        gsum[32 * c:32 * c + 16, :] = _f32(gW1)[16:32, :]
    gw2 = rep4(padc(gW2, 32))
    idn = np.eye(P, dtype=np.float32)

    def bias_col(b, valid=32):
        v = np.zeros((P, 1), np.float32)
        b = _f32(b).ravel()
        for p in range(P):
            r = p % 32
            if r < valid:
                v[p, 0] = b[r % len(b)]
        return v

    parts = [w1, w2, w3, sw1, sw2, sw3, ones_full, ones_excl, gsel, gsum,
             gw2, idn, bias_col(ob1), bias_col(ob2), bias_col(ob3, 16),
             bias_col(sb1), bias_col(sb2), bias_col(sb3, 16),
             bias_col(gb1), bias_col(gb2, 2)]
    wp = np.concatenate(parts, axis=1)
    assert wp.shape == (P, WCOLS), wp.shape
    return wp, (sel_p, sel_i)


def _build_nc(sel_p, sel_i, sel_row, no_tail=False):
    import concourse.bacc as bacc
    import concourse.mybir as mybir
    from concourse.tile import TileContext

    f32 = mybir.dt.float32
    Relu = mybir.ActivationFunctionType.Relu
    Copy = mybir.ActivationFunctionType.Copy
    Ident = mybir.ActivationFunctionType.Identity
    add_op = mybir.AluOpType.add
    max_op = mybir.AluOpType.max

    nc = bacc.Bacc("TRN2", target_bir_lowering=False, debug=False,
                   num_devices=N_CORES)
    x_ext = nc.dram_tensor("x", [B_C, A * D], f32, kind="ExternalInput").ap()
    w_ext = nc.dram_tensor("wpack", [128, WCOLS], f32, kind="ExternalInput").ap()
    o_ext = nc.dram_tensor("out", [2, B_C], f32, kind="ExternalOutput").ap()

    with TileContext(nc) as tc:
        with (
            tc.tile_pool(name="const", bufs=1) as cpool,
            tc.tile_pool(name="xin", bufs=3) as xpool,
            tc.tile_pool(name="xt", bufs=SP + 2) as xtpool,
            tc.tile_pool(name="h", bufs=6) as hpool,
            tc.tile_pool(name="z3s", bufs=12) as z3pool,
            tc.tile_pool(name="gl", bufs=2) as glpool,
            tc.tile_pool(name="zp", bufs=5, space="PSUM") as zpool,
            tc.tile_pool(name="tp", bufs=1, space="PSUM") as tpool,
            tc.tile_pool(name="gp", bufs=2, space="PSUM") as gpool,
        ):
            W = cpool.tile([128, WCOLS], f32, name="W")
            nc.sync.dma_start(out=W[:], in_=w_ext[:])

            def bias(off):
                return W[:, off:off + 1]

            def evac(dst, src, boff, func, dve, lo=0, size=128):
                b = W[lo:lo + size, boff:boff + 1]
                if dve:
                    if func == "relu":
                        nc.vector.tensor_scalar(dst, src, b, 0.0,
                                                add_op, max_op)
                    elif func == "add":
                        nc.vector.tensor_scalar_add(dst, src, b)
                    else:
                        nc.vector.tensor_copy(dst, src)
                else:
                    if func == "relu":
                        nc.scalar.activation(dst, src, Relu, bias=b)
                    elif func == "add":
                        nc.scalar.activation(dst, src, Ident, bias=b)
                    else:
                        nc.scalar.activation(dst, src, Copy)

            for chunk in range(CHUNKS):
                b0 = chunk * NCH
                xin = xpool.tile([128, NCH], f32, tag="xin", name=f"xin{chunk}")
                nc.sync.dma_start(
                    out=xin[:].rearrange("p (k f) -> p k f", f=128),
                    in_=x_ext[b0:b0 + NCH, :].rearrange(
                        "(k p) f -> p k f", p=128))
                tband = tpool.tile([128, NCH], f32, tag="tp", name=f"tband{chunk}")
                for k in range(4):
                    nc.tensor.transpose(
                        tband[:, 128 * k:128 * (k + 1)],
                        xin[:, 128 * k:128 * (k + 1)],
                        W[:, OIDN:OIDN + 128])
                xt = xtpool.tile([128, NCH], f32, tag="xt", name=f"xt{chunk}")
                evac(xt[:], tband[:], 0, "copy", dve=(chunk % 2 == 0))

                z3s_c = [None] * 8
                for p in range(2):
                    banks1 = [zpool.tile([128, NCH], f32, tag="z", name=f"z1_{chunk}_{p}_{i}")
                              for i in range(4)]
                    for i in range(4):
                        for j in range(4):
                            u = 4 * p + j
                            nc.tensor.matmul(
                                banks1[i][32 * j:32 * j + 32, :],
                                W[32 * i:32 * i + 32,
                                  OW1BD + 32 * u:OW1BD + 32 * u + 32],
                                xt[32 * i:32 * i + 32, :],
                                start=True, stop=True,
                                tile_position=(32 * i, 32 * j))
                    h1t = []
                    for i in range(4):
                        h1 = hpool.tile([128, NCH], f32, tag="h1", name=f"h1_{chunk}_{p}_{i}")
                        evac(h1[:], banks1[i][:], OB1, "relu", dve=(i % 2 == 0))
                        h1t.append(h1)
                    banks2 = [zpool.tile([128, NCH], f32, tag="z", name=f"z2_{chunk}_{p}_{i}")
                              for i in range(4)]
                    for i in range(4):
                        for j in range(4):
                            nc.tensor.matmul(
                                banks2[j][32 * i:32 * i + 32, :],
                                W[32 * j:32 * j + 32, OW2:OW2 + 32],
                                h1t[i][32 * j:32 * j + 32, :],
                                start=True, stop=True,
                                tile_position=(32 * j, 32 * i))
                    h2t = []
                    for j in range(4):
                        h2 = hpool.tile([128, NCH], f32, tag="h2", name=f"h2_{chunk}_{p}_{j}")
                        evac(h2[:], banks2[j][:], OB2, "relu", dve=(j % 2 == 1))
                        h2t.append(h2)
                    banks3 = [zpool.tile([128, NCH], f32, tag="z", name=f"z3_{chunk}_{p}_{i}")
                              for i in range(4)]
                    for j in range(4):
                        for i in range(4):
                            nc.tensor.matmul(
                                banks3[i][32 * j:32 * j + 32, :],
                                W[32 * i:32 * i + 32, OW3:OW3 + 32],
                                h2t[j][32 * i:32 * i + 32, :],
                                start=True, stop=True,
                                tile_position=(32 * i, 32 * j))
                    for i in range(4):
                        z3 = z3pool.tile([128, NCH], f32, tag="z3s", name=f"z3s_{chunk}_{p}_{i}")
                        evac(z3[:], banks3[i][:], OB3, "relu", dve=(i % 2 == 0))
                        z3s_c[4 * p + i] = (p, i, z3)

                # ---- per-chunk tail: sel MLP, agent sum, global head ----
                if no_tail:
                    continue
                si = sel_row // 32
                selz1 = gpool.tile([128, NCH], f32, tag="g", name=f"selz1_{chunk}")
                nc.tensor.matmul(
                    selz1[0:32, :],
                    W[32 * si:32 * si + 32, OSW1:OSW1 + 32],
                    xt[32 * si:32 * si + 32, :],
                    start=True, stop=True, tile_position=(32 * si, 0))
                sh1 = glpool.tile([32, NCH], f32, tag="sh1", name=f"sh1_{chunk}")
                evac(sh1[:], selz1[0:32, :], OSB1, "relu", dve=True, size=32)

                selz2 = gpool.tile([128, NCH], f32, tag="g", name=f"selz2_{chunk}")
                nc.tensor.matmul(
                    selz2[0:32, :],
                    W[0:32, OSW2:OSW2 + 32],
                    sh1[0:32, :],
                    start=True, stop=True, tile_position=(0, 0))
                sh2 = glpool.tile([32, NCH], f32, tag="sh2", name=f"sh2_{chunk}")
                evac(sh2[:], selz2[0:32, :], OSB2, "relu", dve=False, size=32)

                selz3 = gpool.tile([128, NCH], f32, tag="g", name=f"selz3_{chunk}")
                nc.tensor.matmul(
                    selz3[0:32, :],
                    W[0:32, OSW3:OSW3 + 32],
                    sh2[0:32, :],
                    start=True, stop=True, tile_position=(0, 0))
                sh3 = glpool.tile([32, NCH], f32, tag="sh3", name=f"sh3_{chunk}")
                evac(sh3[:], selz3[0:32, :], OSB3, "relu", dve=True, size=32)

                # masked agent sum: 8 plain full-array accumulating matmuls
                sumo = gpool.tile([128, NCH], f32, tag="g", name=f"sumo_{chunk}")
                for t in range(8):
                    p, i, z3 = z3s_c[t]
                    oo = OONE if (p == sel_p and i == sel_i) else OONF
                    nc.tensor.matmul(
                        sumo[0:32, :],
                        W[:, oo:oo + 32],
                        z3[:],
                        start=(t == 0), stop=(t == 7))
                sumg = glpool.tile([32, NCH], f32, tag="sumg", name=f"sumg_{chunk}")
                evac(sumg[:], sumo[0:32, :], 0, "copy", dve=(chunk % 2 == 0), size=32)

                # zg = gW1_sel.T @ sel_out + gW1_sum.T @ sum_other (same-bank accum)
                zg = gpool.tile([128, NCH], f32, tag="g", name=f"zg_{chunk}")
                nc.tensor.matmul(
                    zg[0:32, :], W[0:16, OGSEL:OGSEL + 32], sh3[0:16, :],
                    start=True, stop=False)
                nc.tensor.matmul(
                    zg[0:32, :], W[0:16, OGSUM:OGSUM + 32], sumg[0:16, :],
                    start=False, stop=True)
                hg = glpool.tile([32, NCH], f32, tag="hg", name=f"hg_{chunk}")
                evac(hg[:], zg[0:32, :], OGB1, "relu", dve=(chunk % 2 == 1), size=32)

                qp = gpool.tile([128, NCH], f32, tag="g", name=f"qp_{chunk}")
                nc.tensor.matmul(
                    qp[0:32, :], W[0:32, OGW2:OGW2 + 32], hg[0:32, :],
                    start=True, stop=True, tile_position=(0, 0))
                qsb = glpool.tile([32, NCH], f32, tag="q", name=f"qsb_{chunk}")
                evac(qsb[:], qp[0:32, :], OGB2, "add", dve=False, size=32)
                nc.sync.dma_start(out=o_ext[0:1, b0:b0 + NCH], in_=qsb[0:1, :])
                nc.sync.dma_start(out=o_ext[1:2, b0:b0 + NCH], in_=qsb[1:2, :])
    nc.compile()
    return nc


def kernel(**inputs):
    x = _f32(inputs["joint_state_actions"])
    sel = int(inputs["selected_agent_idx"])

    wpack, (sel_p, sel_i) = _build_wpack(
        sel,
        inputs["oW1"], inputs["ob1"], inputs["oW2"], inputs["ob2"],
        inputs["oW3"], inputs["ob3"],
        inputs["sW1"], inputs["sb1"], inputs["sW2"], inputs["sb2"],
        inputs["sW3"], inputs["sb3"],
        inputs["gW1"], inputs["gb1"], inputs["gW2"], inputs["gb2"])

    key = (sel_p, sel_i, sel)
    if key not in _COMPILED:
        _COMPILED[key] = _build_nc(sel_p, sel_i, 4 * sel)
    nc = _COMPILED[key]

    from concourse.bass_utils import run_bass_kernel_spmd
    shards = [np.ascontiguousarray(x[i * B_C:(i + 1) * B_C])
              for i in range(N_CORES)]
    in_maps = [{"x": s, "wpack": wpack} for s in shards]
    res = run_bass_kernel_spmd(nc, in_maps, list(range(N_CORES)),
                               trace=TRACE, **TRACE_KW)
    global LAST_RESULT
    LAST_RESULT = res

    q01 = np.concatenate([res.results[i]["out"] for i in range(N_CORES)],
                         axis=1)
    act = np.clip(x[:, 4 * sel + 3].astype(np.int32), 0, 1)
    out = np.where(act == 0, q01[0], q01[1]).astype(np.float32)
    return out[:, None]



# revision 8
# speedup vs baseline: 2.6139x; 2.6139x over previous
"""Trainium2 Bass kernel for DecomposedQValueNN (gnn_message_passing).

Per batch row b of x[65536, 128]:
  xa = x.reshape(B, 32, 4); other_a = MLP_o(xa[:,a]) (3 relu layers, 4-32-32-16)
  sum_other = sum_{a != sel} other_a;  sel_out = MLP_s(xa[:,sel])
  h = relu([sel_out; sum_other] @ gW1 + gb1); q = h @ gW2 + gb2
  out[b] = q[b, clip(int(xa[b,sel,3]),0,1)]

Device mapping (8 cores, pure batch data-parallel, 8192 rows/core):
  activations transposed [feat, batch]; PE 32x32 array tiling runs 16
  per-agent matmuls concurrently in float32r (1 cycle/row). The sel-agent
  MLP is fused into the per-agent pipeline by patching that agent's weight
  blocks (its other-MLP output is unused). L3 outputs pack two passes
  hi/lo into 4 dense PSUM banks. The agent sum + global first layer
  collapse into 4 accumulating K=128 matmuls with composite G matrices
  (gW1 blocks, sel/other routing, masking baked in). Bias+relu fused into
  PSUM->SBUF evacuation rotated across Scalar/Vector/GpSimd engines.
  Final 2-way q gather on host.
"""

import numpy as np

B_FULL = 65536
N_CORES = 8
B_C = B_FULL // N_CORES       # 8192
A, D = 32, 4
NCH = 512                     # batch cols per chunk (1 PSUM bank fp32)
CHUNKS = B_C // NCH           # 16

# fp16 weight tensor column offsets
OW1 = 0                       # [128, 256] L1 block-diag (sel patched)
OW2 = 256                     # [128, 32] L2 shared
OSW2 = 288                    # [128, 32] L2 sel
OW3LO = 320                   # [128, 32] L3 pass0 -> cols 0..15
OW3HI = 352                   # [128, 32] L3 pass1 -> cols 16..31
OSW3X = 384                   # [128, 32] L3 sel (lo or hi per sel pass)
OG = 416                      # [128, 128] G_i composite, i=0..3
OGW2 = 544                    # [128, 2]  gW2
WHCOLS = 546
# fp32 aux tensor column offsets (identity + bias columns)
OIDN = 0                      # [128, 128] identity (transpose)
OB1N, OB1S = 128, 129         # bias cols
OB2N, OB2S = 130, 131
OB3N, OB3S = 132, 133
OBG1, OBG2 = 134, 135
WFCOLS = 136

_COMPILED = {}

# test-harness knobs (harness leaves these at defaults)
TRACE = False
TRACE_KW = {}
LAST_RESULT = None


def _f32(a):
    return np.ascontiguousarray(a, dtype=np.float32)


def _build_wpack(sel, oW1, ob1, oW2, ob2, oW3, ob3,
                 sW1, sb1, sW2, sb2, sW3, sb3, gW1, gb1, gW2, gb2):
    P = 128
    i_s, u_s = sel // 8, sel % 8
    p_s, j_s = u_s // 4, u_s % 4

    def rep4(w):              # [32, m] -> [128, m]
        return np.tile(_f32(w), (4, 1))

    # L1: block u (=4p+j) has the agent's [4,32] weights at rows 4u..4u+3
    # within each 32-row K strip; strip i serves agents 8i+u.
    w1bd = np.zeros((32, 8 * 32), np.float32)
    for u in range(8):
        w1bd[4 * u:4 * u + 4, 32 * u:32 * u + 32] = _f32(oW1)
    w1 = np.tile(w1bd, (4, 1))
    w1[32 * i_s + 4 * u_s:32 * i_s + 4 * u_s + 4,
       32 * u_s:32 * u_s + 32] = _f32(sW1)

    w2 = rep4(oW2)
    sw2 = rep4(sW2)

    def padlo(w):
        return np.concatenate([_f32(w), np.zeros((32, 16), np.float32)], 1)

    def padhi(w):
        return np.concatenate([np.zeros((32, 16), np.float32), _f32(w)], 1)

    w3lo = rep4(padlo(oW3))
    w3hi = rep4(padhi(oW3))
    sw3x = rep4(padhi(sW3) if p_s == 1 else padlo(sW3))

    # G_i[32j+16p+m, n] = gW1[16+m, n] for agent a=8i+4p+j != sel,
    #                     gW1[m, n]    for a == sel (sel_out block).
    gW1 = _f32(gW1)
    G = np.zeros((P, 4 * 32), np.float32)
    for i in range(4):
        for p in range(2):
            for j in range(4):
                a = 8 * i + 4 * p + j
                r0 = 32 * j + 16 * p
                blk = gW1[0:16, :] if a == sel else gW1[16:32, :]
                G[r0:r0 + 16, 32 * i:32 * i + 32] = blk

    gw2 = np.zeros((P, 2), np.float32)
    gw2[0:32, :] = _f32(gW2)
    idn = np.eye(P, dtype=np.float32)

    def col(fn):
        v = np.zeros((P, 1), np.float32)
        for r in range(P):
            v[r, 0] = fn(r)
        return v

    ob1, ob2, ob3 = _f32(ob1).ravel(), _f32(ob2).ravel(), _f32(ob3).ravel()
    sb1, sb2, sb3 = _f32(sb1).ravel(), _f32(sb2).ravel(), _f32(sb3).ravel()
    gb1, gb2 = _f32(gb1).ravel(), _f32(gb2).ravel()

    b1n = col(lambda r: ob1[r % 32])
    b1s = b1n.copy()
    b1s[32 * j_s:32 * j_s + 32, 0] = sb1          # bank1[i_s], pass p_s
    b2n = col(lambda r: ob2[r % 32])
    b2s = b2n.copy()
    b2s[32 * i_s:32 * i_s + 32, 0] = sb2          # bank2[j_s], pass p_s
    b3n = col(lambda r: ob3[r % 16])
    b3s = b3n.copy()
    b3s[32 * j_s + 16 * p_s:32 * j_s + 16 * p_s + 16, 0] = sb3  # bank3[i_s]
    bg1 = col(lambda r: gb1[r % 32])
    bg2 = np.zeros((P, 1), np.float32)
    bg2[0:2, 0] = gb2

    wph = np.concatenate([w1, w2, sw2, w3lo, w3hi, sw3x, G, gw2],
                         axis=1).astype(np.float16)
    wpf = np.concatenate([idn, b1n, b1s, b2n, b2s, b3n, b3s, bg1, bg2],
                         axis=1)
    assert wph.shape == (P, WHCOLS) and wpf.shape == (P, WFCOLS)
    return wph, wpf, (p_s, i_s, j_s)


def _build_nc(p_s, i_s, j_s):
    import concourse.bacc as bacc
    import concourse.mybir as mybir
    from concourse.tile import TileContext

    f32 = mybir.dt.float32
    f32r = mybir.dt.float32r
    f16 = mybir.dt.float16
    Relu = mybir.ActivationFunctionType.Relu
    Ident = mybir.ActivationFunctionType.Identity
    Copy = mybir.ActivationFunctionType.Copy
    add_op = mybir.AluOpType.add
    max_op = mybir.AluOpType.max

    nc = bacc.Bacc("TRN2", target_bir_lowering=False, debug=False,
                   num_devices=N_CORES)
    x_ext = nc.dram_tensor("x", [B_C, A * D], f32r, kind="ExternalInput").ap()
    wh_ext = nc.dram_tensor("wpackh", [128, WHCOLS], f16,
                            kind="ExternalInput").ap()
    wf_ext = nc.dram_tensor("wpackf", [128, WFCOLS], f32r,
                            kind="ExternalInput").ap()
    o_ext = nc.dram_tensor("out", [2, B_C], f32, kind="ExternalOutput").ap()

    with TileContext(nc) as tc:
        with (
            tc.tile_pool(name="const", bufs=1) as cpool,
            tc.tile_pool(name="xin", bufs=3) as xpool,
            tc.tile_pool(name="xt", bufs=3) as xtpool,
            tc.tile_pool(name="h1", bufs=6) as h1pool,
            tc.tile_pool(name="h2", bufs=9) as h2pool,
            tc.tile_pool(name="z3", bufs=5) as z3pool,
            tc.tile_pool(name="gl", bufs=4) as glpool,
            tc.tile_pool(name="zp", bufs=5, space="PSUM") as zpool,
            tc.tile_pool(name="tp", bufs=1, space="PSUM") as tpool,
            tc.tile_pool(name="gp", bufs=2, space="PSUM") as gpool,
        ):
            W = cpool.tile([128, WHCOLS], f16, name="W")
            nc.sync.dma_start(out=W[:], in_=wh_ext[:])
            Wf = cpool.tile([128, WFCOLS], f32r, name="Wf")
            nc.sync.dma_start(out=Wf[:], in_=wf_ext[:])

            # evac engine rotation (GpSimd cannot access PSUM)
            rot = {"n": 0}
            pat = "svsvsvs"  # 4 scalar : 3 vector

            def evac(dst, src, boff, func, lo=0, size=128):
                e = pat[rot["n"] % len(pat)]
                rot["n"] += 1
                b = Wf[lo:lo + size, boff:boff + 1].bitcast(f32)
                if e == "s":
                    if func == "relu":
                        nc.scalar.activation(dst, src, Relu, bias=b)
                    elif func == "add":
                        nc.scalar.activation(dst, src, Ident, bias=b)
                    else:
                        nc.scalar.activation(dst, src, Copy)
                else:
                    eng = nc.vector if e == "v" else nc.gpsimd
                    if func == "relu":
                        eng.tensor_scalar(dst, src, b, 0.0, add_op, max_op)
                    elif func == "add":
                        eng.tensor_scalar_add(dst, src, b)
                    else:
                        eng.tensor_copy(dst, src)

            for chunk in range(CHUNKS):
                b0 = chunk * NCH
                xin = xpool.tile([128, NCH], f32r, tag="xin", name=f"xin{chunk}")
                nc.sync.dma_start(
                    out=xin[:].rearrange("p (k f) -> p k f", f=128),
                    in_=x_ext[b0:b0 + NCH, :].rearrange(
                        "(k p) f -> p k f", p=128))
                tband = tpool.tile([128, NCH], f32r, tag="tp", name=f"tband{chunk}")
                for k in range(4):
                    nc.tensor.transpose(
                        tband[:, 128 * k:128 * (k + 1)],
                        xin[:, 128 * k:128 * (k + 1)],
                        Wf[:, OIDN:OIDN + 128])
                xt = xtpool.tile([128, NCH], f16, tag="xt", name=f"xt{chunk}")
                evac(xt[:], tband[:], 0, "copy")

                h2t = [[None] * 4, [None] * 4]
                for p in range(2):
                    banks1 = [zpool.tile([128, NCH], f32, tag="z",
                                         name=f"z1_{chunk}_{p}_{i}")
                              for i in range(4)]
                    for i in range(4):
                        for j in range(4):
                            u = 4 * p + j
                            nc.tensor.matmul(
                                banks1[i][32 * j:32 * j + 32, :],
                                W[32 * i:32 * i + 32,
                                    OW1 + 32 * u:OW1 + 32 * u + 32],
                                xt[32 * i:32 * i + 32, :],
                                start=True, stop=True,
                                tile_position=(32 * i, 32 * j))
                    h1t = []
                    for i in range(4):
                        h1 = h1pool.tile([128, NCH], f16, tag="h1",
                                         name=f"h1_{chunk}_{p}_{i}")
                        boff = OB1S if (p == p_s and i == i_s) else OB1N
                        evac(h1[:], banks1[i][:], boff, "relu")
                        h1t.append(h1)
                    banks2 = [zpool.tile([128, NCH], f32, tag="z",
                                         name=f"z2_{chunk}_{p}_{i}")
                              for i in range(4)]
                    for i in range(4):
                        for j in range(4):
                            sel_q = (p == p_s and i == i_s and j == j_s)
                            wo = OSW2 if sel_q else OW2
                            nc.tensor.matmul(
                                banks2[j][32 * i:32 * i + 32, :],
                                W[32 * j:32 * j + 32, wo:wo + 32],
                                h1t[i][32 * j:32 * j + 32, :],
                                start=True, stop=True,
                                tile_position=(32 * j, 32 * i))
                    for j in range(4):
                        h2 = h2pool.tile([128, NCH], f16, tag="h2",
                                         name=f"h2_{chunk}_{p}_{j}")
                        boff = OB2S if (p == p_s and j == j_s) else OB2N
                        evac(h2[:], banks2[j][:], boff, "relu")
                        h2t[p][j] = h2

                # L3: both passes accumulate hi/lo into 4 dense banks
                banks3 = [zpool.tile([128, NCH], f32, tag="z",
                                     name=f"z3_{chunk}_{i}")
                          for i in range(4)]
                for p in range(2):
                    for j in range(4):
                        for i in range(4):
                            sel_q = (p == p_s and i == i_s and j == j_s)
                            wo = OSW3X if sel_q else (OW3HI if p else OW3LO)
                            nc.tensor.matmul(
                                banks3[i][32 * j:32 * j + 32, :],
                                W[32 * i:32 * i + 32, wo:wo + 32],
                                h2t[p][j][32 * i:32 * i + 32, :],
                                start=(p == 0), stop=(p == 1),
                                tile_position=(32 * i, 32 * j))
                z3t = []
                for i in range(4):
                    z3 = z3pool.tile([128, NCH], f16, tag="z3",
                                     name=f"z3s_{chunk}_{i}")
                    boff = OB3S if i == i_s else OB3N
                    evac(z3[:], banks3[i][:], boff, "relu")
                    z3t.append(z3)

                # zg[0:32] = sum_i G_i.T @ z3[i]  (gW1 + agent sum fused)
                zg = gpool.tile([128, NCH], f32, tag="g", name=f"zg_{chunk}")
                for i in range(4):
                    nc.tensor.matmul(
                        zg[0:32, :],
                        W[:, OG + 32 * i:OG + 32 * i + 32],
                        z3t[i][:],
                        start=(i == 0), stop=(i == 3))
                hg = glpool.tile([32, NCH], f16, tag="hg", name=f"hg_{chunk}")
                evac(hg[:], zg[0:32, :], OBG1, "relu", size=32)

                qp = gpool.tile([128, NCH], f32, tag="g", name=f"qp_{chunk}")
                nc.tensor.matmul(
                    qp[0:2, :], W[0:32, OGW2:OGW2 + 2], hg[0:32, :],
                    start=True, stop=True)
                qsb = glpool.tile([2, NCH], f32, tag="q", name=f"qsb_{chunk}")
                evac(qsb[:], qp[0:2, :], OBG2, "add", size=2)
                nc.sync.dma_start(out=o_ext[0:2, b0:b0 + NCH], in_=qsb[0:2, :])
    nc.compile()
    return nc


def kernel(**inputs):
    x = _f32(inputs["joint_state_actions"])
    sel = int(inputs["selected_agent_idx"])

    wpackh, wpackf, (p_s, i_s, j_s) = _build_wpack(
        sel,
        inputs["oW1"], inputs["ob1"], inputs["oW2"], inputs["ob2"],
        inputs["oW3"], inputs["ob3"],
        inputs["sW1"], inputs["sb1"], inputs["sW2"], inputs["sb2"],
        inputs["sW3"], inputs["sb3"],
        inputs["gW1"], inputs["gb1"], inputs["gW2"], inputs["gb2"])

    key = (p_s, i_s, j_s)
    if key not in _COMPILED:
        _COMPILED[key] = _build_nc(p_s, i_s, j_s)
    nc = _COMPILED[key]

    from concourse.bass_utils import run_bass_kernel_spmd
    shards = [np.ascontiguousarray(x[i * B_C:(i + 1) * B_C])
              for i in range(N_CORES)]
    in_maps = [{"x": s, "wpackh": wpackh, "wpackf": wpackf}
               for s in shards]
    res = run_bass_kernel_spmd(nc, in_maps, list(range(N_CORES)),
                               trace=TRACE, **TRACE_KW)
    global LAST_RESULT
    LAST_RESULT = res

    q01 = np.concatenate([res.results[i]["out"] for i in range(N_CORES)],
                         axis=1)
    act = np.clip(x[:, 4 * sel + 3].astype(np.int32), 0, 1)
    out = np.where(act == 0, q01[0], q01[1]).astype(np.float32)
    return out[:, None]


# revision 10
# speedup vs baseline: 2.7362x; 1.0468x over previous
"""Trainium2 Bass kernel for DecomposedQValueNN (gnn_message_passing).

Per batch row b of x[65536, 128]:
  xa = x.reshape(B, 32, 4); other_a = MLP_o(xa[:,a]) (3 relu layers, 4-32-32-16)
  sum_other = sum_{a != sel} other_a;  sel_out = MLP_s(xa[:,sel])
  h = relu([sel_out; sum_other] @ gW1 + gb1); q = h @ gW2 + gb2
  out[b] = q[b, clip(int(xa[b,sel,3]),0,1)]

Device mapping (8 cores, pure batch data-parallel, 8192 rows/core):
  activations transposed [feat, batch]; PE 32x32 array tiling runs 16
  per-agent matmuls concurrently in float32r (1 cycle/row). The sel-agent
  MLP is fused into the per-agent pipeline by patching that agent's weight
  blocks (its other-MLP output is unused). L3 outputs pack two passes
  hi/lo into 4 dense PSUM banks. The agent sum + global first layer
  collapse into 4 accumulating K=128 matmuls with composite G matrices
  (gW1 blocks, sel/other routing, masking baked in). Bias+relu fused into
  PSUM->SBUF evacuation rotated across Scalar/Vector/GpSimd engines.
  Final 2-way q gather on host.
"""

import numpy as np

B_FULL = 65536
N_CORES = 8
B_C = B_FULL // N_CORES       # 8192
A, D = 32, 4
NCH = 512                     # batch cols per chunk (1 PSUM bank fp32)
CHUNKS = B_C // NCH           # 16

# fp16 weight tensor column offsets
OW1 = 0                       # [128, 256] L1 block-diag (sel patched)
OW2 = 256                     # [128, 32] L2 shared
OSW2 = 288                    # [128, 32] L2 sel
OW3LO = 320                   # [128, 32] L3 pass0 -> cols 0..15
OW3HI = 352                   # [128, 32] L3 pass1 -> cols 16..31
OSW3X = 384                   # [128, 32] L3 sel (lo or hi per sel pass)
OG = 416                      # [128, 128] G_i composite, i=0..3
OGW2 = 544                    # [128, 2]  gW2
WHCOLS = 546
# fp32 aux tensor column offsets (identity + bias columns)
OIDN = 0                      # [128, 128] identity (transpose)
OB1N, OB1S = 128, 129         # bias cols
OB2N, OB2S = 130, 131
OB3N, OB3S = 132, 133
OBG1, OBG2 = 134, 135
WFCOLS = 136

_COMPILED = {}

# test-harness knobs (harness leaves these at defaults)
TRACE = False
TRACE_KW = {}
LAST_RESULT = None


def _f32(a):
    return np.ascontiguousarray(a, dtype=np.float32)


def _build_wpack(sel, oW1, ob1, oW2, ob2, oW3, ob3,
                 sW1, sb1, sW2, sb2, sW3, sb3, gW1, gb1, gW2, gb2):
    P = 128
    i_s, u_s = sel // 8, sel % 8
    p_s, j_s = u_s // 4, u_s % 4

    def rep4(w):              # [32, m] -> [128, m]
        return np.tile(_f32(w), (4, 1))

    # L1: block u (=4p+j) has the agent's [4,32] weights at rows 4u..4u+3
    # within each 32-row K strip; strip i serves agents 8i+u.
    w1bd = np.zeros((32, 8 * 32), np.float32)
    for u in range(8):
        w1bd[4 * u:4 * u + 4, 32 * u:32 * u + 32] = _f32(oW1)
    w1 = np.tile(w1bd, (4, 1))
    w1[32 * i_s + 4 * u_s:32 * i_s + 4 * u_s + 4,
       32 * u_s:32 * u_s + 32] = _f32(sW1)

    w2 = rep4(oW2)
    sw2 = rep4(sW2)

    def padlo(w):
        return np.concatenate([_f32(w), np.zeros((32, 16), np.float32)], 1)

    def padhi(w):
        return np.concatenate([np.zeros((32, 16), np.float32), _f32(w)], 1)

    w3lo = rep4(padlo(oW3))
    w3hi = rep4(padhi(oW3))
    sw3x = rep4(padhi(sW3) if p_s == 1 else padlo(sW3))

    # G_i[32j+16p+m, n] = gW1[16+m, n] for agent a=8i+4p+j != sel,
    #                     gW1[m, n]    for a == sel (sel_out block).
    gW1 = _f32(gW1)
    G = np.zeros((P, 4 * 32), np.float32)
    for i in range(4):
        for p in range(2):
            for j in range(4):
                a = 8 * i + 4 * p + j
                r0 = 32 * j + 16 * p
                blk = gW1[0:16, :] if a == sel else gW1[16:32, :]
                G[r0:r0 + 16, 32 * i:32 * i + 32] = blk

    gw2 = rep4(gW2)
    idn = np.eye(P, dtype=np.float32)

    def col(fn):
        v = np.zeros((P, 1), np.float32)
        for r in range(P):
            v[r, 0] = fn(r)
        return v

    ob1, ob2, ob3 = _f32(ob1).ravel(), _f32(ob2).ravel(), _f32(ob3).ravel()
    sb1, sb2, sb3 = _f32(sb1).ravel(), _f32(sb2).ravel(), _f32(sb3).ravel()
    gb1, gb2 = _f32(gb1).ravel(), _f32(gb2).ravel()

    b1n = col(lambda r: ob1[r % 32])
    b1s = b1n.copy()
    b1s[32 * j_s:32 * j_s + 32, 0] = sb1          # bank1[i_s], pass p_s
    b2n = col(lambda r: ob2[r % 32])
    b2s = b2n.copy()
    b2s[32 * i_s:32 * i_s + 32, 0] = sb2          # bank2[j_s], pass p_s
    b3n = col(lambda r: ob3[r % 16])
    b3s = b3n.copy()
    b3s[32 * j_s + 16 * p_s:32 * j_s + 16 * p_s + 16, 0] = sb3  # bank3[i_s]
    bg1 = col(lambda r: gb1[r % 32])
    bg2 = np.zeros((P, 1), np.float32)
    for c in range(4):
        bg2[32 * c, 0] = gb2[0]
        bg2[32 * c + 1, 0] = gb2[1]

    wph = np.concatenate([w1, w2, sw2, w3lo, w3hi, sw3x, G, gw2],
                         axis=1).astype(np.float16)
    wpf = np.concatenate([idn, b1n, b1s, b2n, b2s, b3n, b3s, bg1, bg2],
                         axis=1)
    assert wph.shape == (P, WHCOLS) and wpf.shape == (P, WFCOLS)
    return wph, wpf, (p_s, i_s, j_s)


def _build_nc(p_s, i_s, j_s):
    import concourse.bacc as bacc
    import concourse.mybir as mybir
    from concourse.tile import TileContext

    f32 = mybir.dt.float32
    f32r = mybir.dt.float32r
    f16 = mybir.dt.float16
    Relu = mybir.ActivationFunctionType.Relu
    Ident = mybir.ActivationFunctionType.Identity
    Copy = mybir.ActivationFunctionType.Copy
    add_op = mybir.AluOpType.add
    max_op = mybir.AluOpType.max

    nc = bacc.Bacc("TRN2", target_bir_lowering=False, debug=False,
                   num_devices=N_CORES)
    x_ext = nc.dram_tensor("x", [B_C, A * D], f32r, kind="ExternalInput").ap()
    wh_ext = nc.dram_tensor("wpackh", [128, WHCOLS], f16,
                            kind="ExternalInput").ap()
    wf_ext = nc.dram_tensor("wpackf", [128, WFCOLS], f32r,
                            kind="ExternalInput").ap()
    o_ext = nc.dram_tensor("out", [2, B_C], f32, kind="ExternalOutput").ap()

    with TileContext(nc) as tc:
        with (
            tc.tile_pool(name="const", bufs=1) as cpool,
            tc.tile_pool(name="xin", bufs=3) as xpool,
            tc.tile_pool(name="xt", bufs=3) as xtpool,
            tc.tile_pool(name="h1", bufs=6) as h1pool,
            tc.tile_pool(name="h2", bufs=9) as h2pool,
            tc.tile_pool(name="z3", bufs=5) as z3pool,
            tc.tile_pool(name="gl", bufs=4) as glpool,
            tc.tile_pool(name="zp", bufs=5, space="PSUM") as zpool,
            tc.tile_pool(name="tp", bufs=1, space="PSUM") as tpool,
            tc.tile_pool(name="gp", bufs=2, space="PSUM") as gpool,
        ):
            W = cpool.tile([128, WHCOLS], f16, name="W")
            nc.sync.dma_start(out=W[:], in_=wh_ext[:])
            Wf = cpool.tile([128, WFCOLS], f32r, name="Wf")
            nc.sync.dma_start(out=Wf[:], in_=wf_ext[:])

            # evac engine rotation (GpSimd cannot access PSUM)
            rot = {"n": 0}
            pat = "svsvsvs"  # 4 scalar : 3 vector

            def evac(dst, src, boff, func, lo=0, size=128):
                e = pat[rot["n"] % len(pat)]
                rot["n"] += 1
                b = Wf[lo:lo + size, boff:boff + 1].bitcast(f32)
                if e == "s":
                    if func == "relu":
                        nc.scalar.activation(dst, src, Relu, bias=b)
                    elif func == "add":
                        nc.scalar.activation(dst, src, Ident, bias=b)
                    else:
                        nc.scalar.activation(dst, src, Copy)
                else:
                    eng = nc.vector if e == "v" else nc.gpsimd
                    if func == "relu":
                        eng.tensor_scalar(dst, src, b, 0.0, add_op, max_op)
                    elif func == "add":
                        eng.tensor_scalar_add(dst, src, b)
                    else:
                        eng.tensor_copy(dst, src)

            zgb = qb = None
            for chunk in range(CHUNKS):
                b0 = chunk * NCH
                cg = chunk % 4
                if cg == 0:
                    zgb = gpool.tile([128, NCH], f32, tag="g",
                                     name=f"zgb_{chunk}")
                    qb = gpool.tile([128, NCH], f32, tag="g",
                                    name=f"qb_{chunk}")
                xin = xpool.tile([128, NCH], f32r, tag="xin", name=f"xin{chunk}")
                nc.sync.dma_start(
                    out=xin[:].rearrange("p (k f) -> p k f", f=128),
                    in_=x_ext[b0:b0 + NCH, :].rearrange(
                        "(k p) f -> p k f", p=128))
                tband = tpool.tile([128, NCH], f32r, tag="tp", name=f"tband{chunk}")
                for k in range(4):
                    nc.tensor.transpose(
                        tband[:, 128 * k:128 * (k + 1)],
                        xin[:, 128 * k:128 * (k + 1)],
                        Wf[:, OIDN:OIDN + 128])
                xt = xtpool.tile([128, NCH], f16, tag="xt", name=f"xt{chunk}")
                evac(xt[:], tband[:], 0, "copy")

                h2t = [[None] * 4, [None] * 4]
                for p in range(2):
                    banks1 = [zpool.tile([128, NCH], f32, tag="z",
                                         name=f"z1_{chunk}_{p}_{i}")
                              for i in range(4)]
                    for i in range(4):
                        for j in range(4):
                            u = 4 * p + j
                            nc.tensor.matmul(
                                banks1[i][32 * j:32 * j + 32, :],
                                W[32 * i:32 * i + 32,
                                    OW1 + 32 * u:OW1 + 32 * u + 32],
                                xt[32 * i:32 * i + 32, :],
                                start=True, stop=True,
                                tile_position=(32 * i, 32 * j))
                    h1t = []
                    for i in range(4):
                        h1 = h1pool.tile([128, NCH], f16, tag="h1",
                                         name=f"h1_{chunk}_{p}_{i}")
                        boff = OB1S if (p == p_s and i == i_s) else OB1N
                        evac(h1[:], banks1[i][:], boff, "relu")
                        h1t.append(h1)
                    banks2 = [zpool.tile([128, NCH], f32, tag="z",
                                         name=f"z2_{chunk}_{p}_{i}")
                              for i in range(4)]
                    for i in range(4):
                        for j in range(4):
                            sel_q = (p == p_s and i == i_s and j == j_s)
                            wo = OSW2 if sel_q else OW2
                            nc.tensor.matmul(
                                banks2[j][32 * i:32 * i + 32, :],
                                W[32 * j:32 * j + 32, wo:wo + 32],
                                h1t[i][32 * j:32 * j + 32, :],
                                start=True, stop=True,
                                tile_position=(32 * j, 32 * i))
                    for j in range(4):
                        h2 = h2pool.tile([128, NCH], f16, tag="h2",
                                         name=f"h2_{chunk}_{p}_{j}")
                        boff = OB2S if (p == p_s and j == j_s) else OB2N
                        evac(h2[:], banks2[j][:], boff, "relu")
                        h2t[p][j] = h2

                # L3: both passes accumulate hi/lo into 4 dense banks
                banks3 = [zpool.tile([128, NCH], f32, tag="z",
                                     name=f"z3_{chunk}_{i}")
                          for i in range(4)]
                for p in range(2):
                    for j in range(4):
                        for i in range(4):
                            sel_q = (p == p_s and i == i_s and j == j_s)
                            wo = OSW3X if sel_q else (OW3HI if p else OW3LO)
                            nc.tensor.matmul(
                                banks3[i][32 * j:32 * j + 32, :],
                                W[32 * i:32 * i + 32, wo:wo + 32],
                                h2t[p][j][32 * i:32 * i + 32, :],
                                start=(p == 0), stop=(p == 1),
                                tile_position=(32 * i, 32 * j))
                z3t = []
                for i in range(4):
                    z3 = z3pool.tile([128, NCH], f16, tag="z3",
                                     name=f"z3s_{chunk}_{i}")
                    boff = OB3S if i == i_s else OB3N
                    evac(z3[:], banks3[i][:], boff, "relu")
                    z3t.append(z3)

                # zg[32cg:32cg+32] = sum_i G_i.T @ z3[i] (gW1 + agent sum)
                for i in range(4):
                    nc.tensor.matmul(
                        zgb[32 * cg:32 * cg + 32, :],
                        W[:, OG + 32 * i:OG + 32 * i + 32],
                        z3t[i][:],
                        start=(i == 0), stop=(i == 3),
                        tile_position=(0, 32 * cg))
                if cg == 3:
                    # batched global head for the 4-chunk group
                    hg = glpool.tile([128, NCH], f16, tag="hg",
                                     name=f"hg_{chunk}")
                    evac(hg[:], zgb[:], OBG1, "relu")
                    for c in range(4):
                        nc.tensor.matmul(
                            qb[32 * c:32 * c + 2, :],
                            W[32 * c:32 * c + 32, OGW2:OGW2 + 2],
                            hg[32 * c:32 * c + 32, :],
                            start=True, stop=True,
                            tile_position=(32 * c, 32 * c))
                    qsb = glpool.tile([128, NCH], f32, tag="q",
                                      name=f"qsb_{chunk}")
                    evac(qsb[:], qb[:], OBG2, "add")
                    for c in range(4):
                        bc = (chunk - 3 + c) * NCH
                        nc.sync.dma_start(
                            out=o_ext[0:2, bc:bc + NCH],
                            in_=qsb[32 * c:32 * c + 2, :])
    nc.compile()
    return nc


def kernel(**inputs):
    x = _f32(inputs["joint_state_actions"])
    sel = int(inputs["selected_agent_idx"])

    wpackh, wpackf, (p_s, i_s, j_s) = _build_wpack(
        sel,
        inputs["oW1"], inputs["ob1"], inputs["oW2"], inputs["ob2"],
        inputs["oW3"], inputs["ob3"],
        inputs["sW1"], inputs["sb1"], inputs["sW2"], inputs["sb2"],
        inputs["sW3"], inputs["sb3"],
        inputs["gW1"], inputs["gb1"], inputs["gW2"], inputs["gb2"])

    key = (p_s, i_s, j_s)
    if key not in _COMPILED:
        _COMPILED[key] = _build_nc(p_s, i_s, j_s)
    nc = _COMPILED[key]

    from concourse.bass_utils import run_bass_kernel_spmd
    shards = [np.ascontiguousarray(x[i * B_C:(i + 1) * B_C])
              for i in range(N_CORES)]
    in_maps = [{"x": s, "wpackh": wpackh, "wpackf": wpackf}
               for s in shards]
    res = run_bass_kernel_spmd(nc, in_maps, list(range(N_CORES)),
                               trace=TRACE, **TRACE_KW)
    global LAST_RESULT
    LAST_RESULT = res

    q01 = np.concatenate([res.results[i]["out"] for i in range(N_CORES)],
                         axis=1)
    act = np.clip(x[:, 4 * sel + 3].astype(np.int32), 0, 1)
    out = np.where(act == 0, q01[0], q01[1]).astype(np.float32)
    return out[:, None]
